# revision 24
# baseline (speedup 1.0000x reference)
"""Single-head masked attention (B=4, S=2048, D=1024, fp32) on 8 TRN2 NeuronCores.

V-sharing variant of the A-fusion + key-packing + bf16 kernel: the value
projection V = x_k Wv^T + bv is split by OUTPUT COLUMNS across the core
pair -- each core projects its 512 dv columns over ALL packed keys (the
lhsT is the same xkt tiles S^T uses, so no extra key input), and the
halves are exchanged with a pair AllGather through 1024-wide DRAM bounce
buffers (narrow rows make the collective 3x slower). The attention output
is then a single PV matmul out[q,dv] = attnU^T.T @ V * recip[q].

Per-core PE stream: V-dvhalf(72) -> H(128) -> S^T qc0(72) -> sum(9) ->
S^T qc1(72) -> sum(9) -> PV(144) = 506 tile-matmuls (vs 928 baseline).
The V exchange (~34us) runs first on the PE so it hides under the whole
H + S^T span. PV consumes RAW exp weights; normalization happens on the
PSUM->SBUF move as a per-partition scalar multiply, with the reciprocal
row transposed to per-q-tile columns by tiny PE transpose matmuls.

SPMD asymmetry trick: each core's wvH/bvb inputs hold ITS dv-half of
Wv^T/bv (host-selected), so the device program is identical on all
cores; the AllGather's rank-order concatenation puts dvc0 rows first for
both cores of a pair. bv is folded into V: out_unnorm = sum_k u_k
(V_k + bv) = Z + sumexp*bv, so dividing by sumexp yields out + bv
exactly. Output is [QL, D] directly (no transpose anywhere).

Measured on 8 axon-tunneled TRN2 cores: ~132-134us fresh-process vs
249.6us baseline = 1.87x; rel err 6.891e-3 (gate 2e-2). 506 tile-matmuls
x 216ns = 109us stream + 6.8us preamble + ~6us DMA-floor startup +
4.3us external periodic stalls + ~5us tail.
"""

from contextlib import ExitStack

import ml_dtypes
import numpy as np

import concourse.bacc as bacc
import concourse.mybir as mybir
import concourse.tile as tile
from concourse.bass_utils import run_bass_kernel_spmd

D = 1024       # model dim = head dim
QL = 1024      # queries per core
N_CORES = 8
SCALE = 1.0 / 32.0   # 1/sqrt(D)
MASK_NEG = -30000.0

F32 = mybir.dt.float32
BF16 = mybir.dt.bfloat16
AF = mybir.ActivationFunctionType
NP_BF16 = ml_dtypes.bfloat16


def _build_nc(K_T):
    n_half = (K_T + 1) // 2
    nc = bacc.Bacc(None)

    aP = nc.declare_dram_parameter("aP", [8, 128, D], BF16, isOutput=False)[:]
    xqT = nc.declare_dram_parameter("xqT", [D, QL], BF16, isOutput=False)[:]
    xkP = nc.declare_dram_parameter("xkP", [K_T, 128, D], BF16,
                                    isOutput=False)[:]
    wvH = nc.declare_dram_parameter("wvH", [D, 512], BF16, isOutput=False)[:]
    bvb = nc.declare_dram_parameter("bvb", [128, 512], BF16,
                                    isOutput=False)[:]
    cst = nc.declare_dram_parameter("cst", [128, 16 + K_T], F32,
                                    isOutput=False)[:]
    onesd = nc.declare_dram_parameter("onesd", [128, 2], BF16,
                                      isOutput=False)[:]
    out_d = nc.declare_dram_parameter("out", [QL, D], F32, isOutput=True)[:]

    with tile.TileContext(nc) as tc:
        _emit(nc, tc, K_T, aP, xqT, xkP, wvH, bvb, cst, onesd, out_d)
    nc.finalize()
    return nc


def _emit(nc, tc, K_T, aP, xqT, xkP, wvH, bvb, cst, onesd, out_d):
    with ExitStack() as ctx:
        consts = ctx.enter_context(tc.tile_pool(name="consts", bufs=1))
        hpool = ctx.enter_context(tc.tile_pool(name="h", bufs=8))
        ht = [hpool.tile([128, QL], BF16, tag="ht", name=f"ht{m}")
              for m in range(8)]
        xktpool = ctx.enter_context(tc.tile_pool(name="xkt", bufs=K_T))
        vpool = ctx.enter_context(tc.tile_pool(name="v", bufs=4))
        dram = ctx.enter_context(tc.tile_pool(name="dram", bufs=2,
                                              space="DRAM"))
        pps = ctx.enter_context(tc.tile_pool(name="ps", bufs=6, space="PSUM"))

        # V' dv-half accumulator (all kt, my 512 dv cols), gathered V.
        v_sb = vpool.tile([128, K_T, 512], BF16, tag="vsb", name="v_sb")
        vg = vpool.tile([128, K_T, D], BF16, tag="vg", name="vg")
        # CC bounce buffers stay 1024-wide (narrow rows made the
        # AllGather 3x slower); rows pair two 512-col chunks.
        vin = dram.tile([K_T * 64, D], BF16)
        vout = dram.tile([2 * K_T * 64, D], BF16)

        # ---------------- Phase 1: H = A^T xq^T + a ----------------
        with tc.tile_pool(name="proj", bufs=1) as pp:
            am = [pp.tile([128, 8, 128], BF16, tag="am", bufs=8,
                          name=f"am{m}") for m in range(8)]
            xq4 = [[None] * 2 for _ in range(2)]  # [g][qc], ec = 4g..4g+3
            for g in range(2):
                for qc in range(2):
                    xq4[g][qc] = pp.tile([128, 4, 512], BF16, tag="xq",
                                         bufs=4, name=f"xq{g}_{qc}")

            def ld_xq(eng, g, qc):
                return eng.dma_start(
                    out=xq4[g][qc],
                    in_=xqT[g * 512:(g + 1) * 512,
                            qc * 512:(qc + 1) * 512]
                    .rearrange("(a p) q -> p a q", p=128))

            # V lhsT == S^T lhsT (the xkt tiles), so xkt streams FIRST;
            # wv is only MY dv-half (host-selected -> SPMD asymmetry).
            # Gate for V group 0: xkt0+wvh0 on sync || wvh1 on scalar.
            wvh = [pp.tile([128, 4, 512], BF16, tag="wvh", bufs=2,
                           name=f"wvh{g}") for g in range(2)]
            xkt = []
            for kt in range(K_T):
                xkt.append(xktpool.tile([128, 8, 128], BF16, tag="xkt",
                                        bufs=K_T, name=f"xkt{kt}"))

            def ld_wvh(eng, g):
                return eng.dma_start(
                    out=wvh[g],
                    in_=wvH[g * 512:(g + 1) * 512, :]
                    .rearrange("(a p) d -> p a d", p=128))

            def ld_xkt(kt):
                return nc.sync.dma_start(
                    out=xkt[kt],
                    in_=xkP[kt].rearrange("p (a c) -> p a c", a=8))

            bvb_sb = consts.tile([128, 512], BF16, tag="bvb", name="bvb_sb")
            ld_xkt(0)
            ld_wvh(nc.scalar, 1)
            ld_wvh(nc.sync, 0)
            nc.scalar.dma_start(out=bvb_sb, in_=bvb)
            ld_xkt(1)
            ld_xkt(2)
            ld_xkt(3)
            nc.sync.dma_start(
                out=am[0], in_=aP[0].rearrange("p (a c) -> p a c", a=8))
            ld_xq(nc.scalar, 0, 0)
            ld_xq(nc.scalar, 1, 0)
            for kt in range(4, K_T):
                ld_xkt(kt)
            for m in range(1, 8):
                nc.sync.dma_start(
                    out=am[m], in_=aP[m].rearrange("p (a c) -> p a c", a=8))
            ld_xq(nc.scalar, 0, 1)
            xq_last_dma = ld_xq(nc.scalar, 1, 1)

            def xq_slice(ec, qc):
                return xq4[ec // 4][qc][:, ec % 4, :]
            cst_sb = consts.tile([128, 16 + K_T], F32, tag="cst",
                                 name="cst_sb")
            aCol_sb = cst_sb[:, 0:8]
            mb_sb = cst_sb[:, 16:16 + K_T]
            nc.scalar.dma_start(out=cst_sb, in_=cst)
            ones_sb = consts.tile([128, 2], BF16, tag="ones", name="ones_sb")
            nc.scalar.dma_start(out=ones_sb, in_=onesd)
            warm = consts.tile([128, 2], F32, tag="warm", name="warm")
            nc.scalar.activation(warm, ones_sb, AF.Exp)

            # ---- V' dv-half FIRST on the PE (lhsT = the S^T xkt tiles;
            # each core projects its 512 dv columns over ALL kt -- no
            # odd-K_T pad tile). AllGather reassembles [dvc0-rows |
            # dvc1-rows]; two reads recombine the column halves. ----
            for kt in range(K_T):
                ps = pps.tile([128, 512], F32, tag="ps", name=f"psv{kt}")
                for dc in range(8):
                    nc.tensor.matmul(
                        ps, xkt[kt][:, dc, :],
                        wvh[dc // 4][:, dc % 4, :],
                        start=(dc == 0), stop=(dc == 7))
                nc.vector.tensor_add(v_sb[:, kt, :], ps, bvb_sb)
            nc.gpsimd.dma_start(
                vin[:].rearrange("(a ph) (two d) -> (ph two) a d",
                                 ph=64, two=2), v_sb[:])
            nc.gpsimd.collective_compute(
                "AllGather", mybir.AluOpType.bypass,
                replica_groups=[[0, 1], [2, 3], [4, 5], [6, 7]],
                ins=[vin.opt()], outs=[vout.opt()])
            for half in range(2):
                nc.gpsimd.dma_start(
                    vg[:, :, half * 512:(half + 1) * 512],
                    vout[half * K_T * 64:(half + 1) * K_T * 64, :]
                    .rearrange("(a ph) (two d) -> (ph two) a d",
                               ph=64, two=2))

            # ---- H groups ----
            for qc in range(2):
                for m in range(8):
                    ps = pps.tile([128, 512], F32, tag="ps",
                                  name=f"psh{qc}_{m}")
                    for ec in range(8):
                        nc.tensor.matmul(
                            ps, am[m][:, ec, :], xq_slice(ec, qc),
                            start=(ec == 0), stop=(ec == 7))
                    nc.vector.tensor_scalar_add(
                        ht[m][:, qc * 512:(qc + 1) * 512], ps,
                        aCol_sb[:, m:m + 1])

        # ---------------- Phase 2 ----------------
        with tc.tile_pool(name="att", bufs=1) as at_p:
            # ---- S^T -> exp; sumexp -> reciprocal -> broadcast ----
            at = [[None] * K_T for _ in range(2)]
            rbs = []
            for qc in range(2):
                for kt in range(K_T):
                    ps = pps.tile([128, 512], F32, tag="ps",
                                  name=f"pss{qc}_{kt}")
                    for dc in range(8):
                        nc.tensor.matmul(
                            ps, xkt[kt][:, dc, :],
                            ht[dc][:, qc * 512:(qc + 1) * 512],
                            start=(dc == 0), stop=(dc == 7))
                    a = at_p.tile([128, 512], BF16, tag="at", bufs=2 * K_T,
                                  name=f"at{qc}_{kt}")
                    nc.scalar.activation(
                        a, ps, AF.Exp,
                        bias=mb_sb[:, kt:kt + 1], scale=SCALE)
                    at[qc][kt] = a

                srow = pps.tile([2, 512], F32, tag="ps_sum", bufs=2,
                                name=f"srow{qc}")
                for kt in range(K_T):
                    nc.tensor.matmul(
                        srow, ones_sb, at[qc][kt],
                        start=(kt == 0), stop=(kt == K_T - 1))
                rrow = at_p.tile([2, 512], F32, tag="rrow", bufs=2,
                                 name=f"rrow{qc}")
                nc.vector.reciprocal(rrow[0:1, :], srow[0:1, :])
                rbs.append(rrow)

            # ---- reciprocal row -> per-q-tile columns (PE transpose);
            # PV then consumes RAW exp weights and normalizes on the
            # PSUM->SBUF move (bv folded into V scales with sumexp, so
            # out = (Z + sumexp*bv)/sumexp is exact). ----
            rcol = at_p.tile([128, 8], F32, tag="rcol", name="rcol")

            one1 = cst_sb[0:1, 8:9]   # host writes 1.0 there

            def emit_rcol(qc):
                for qs in range(4):
                    pst = pps.tile([128, 1], F32, tag="ps",
                                   name=f"pst{qc}_{qs}")
                    nc.tensor.matmul(
                        pst, rbs[qc][0:1, qs * 128:(qs + 1) * 128],
                        one1, is_transpose=True)
                    nc.vector.tensor_copy(rcol[:, qc * 4 + qs:
                                               qc * 4 + qs + 1], pst)

            # ---- PV: out[q,dv] = attnU^T.T @ V' * recip[q] ----
            emit_rcol(0)
            for qt in range(8):
                if qt == 4:
                    emit_rcol(1)
                qc, qs = divmod(qt, 4)
                for dvc in range(2):
                    ps = pps.tile([128, 512], F32, tag="ps",
                                  name=f"pso{qt}_{dvc}")
                    for kt in range(K_T):
                        nc.tensor.matmul(
                            ps, at[qc][kt][:, qs * 128:(qs + 1) * 128],
                            vg[:, kt, dvc * 512:(dvc + 1) * 512],
                            start=(kt == 0), stop=(kt == K_T - 1))
                    o = at_p.tile([128, 512], F32, tag="o", bufs=4,
                                  name=f"o{qt}_{dvc}")
                    weng = nc.scalar if qt < 4 else nc.sync
                    last = qt == 7 and dvc == 1
                    for lo, hi in ([(0, 256), (256, 512)] if last
                                   else [(0, 512)]):
                        nc.vector.tensor_scalar_mul(
                            o[:, lo:hi], ps[:, lo:hi],
                            rcol[:, qt:qt + 1])
                        weng.dma_start(
                            out=out_d[qt * 128:(qt + 1) * 128,
                                      dvc * 512 + lo:dvc * 512 + hi],
                            in_=o[:, lo:hi])


def _prep_inputs(x, mask, Wq, bq, Wk, bk, Wv, bv):
    x = np.ascontiguousarray(np.asarray(x, dtype=np.float32))
    mask = np.asarray(mask, dtype=bool)
    Wq = np.asarray(Wq, dtype=np.float64)
    bq = np.asarray(bq, dtype=np.float64)
    Wk = np.asarray(Wk, dtype=np.float64)
    Wv = np.asarray(Wv, dtype=np.float32)
    bv = np.asarray(bv, dtype=np.float32)
    del bk  # exactly cancelled by softmax shift invariance

    A = (Wq.T @ Wk).astype(np.float32)
    a_vec = (bq @ Wk).astype(np.float32)
    aP = np.ascontiguousarray(
        A.reshape(8, 128, 8, 128).transpose(2, 1, 0, 3)
        .reshape(8, 128, D).astype(NP_BF16))
    wvT_f = Wv.T.astype(NP_BF16)
    aColT = a_vec.reshape(8, 128).T
    bvT = bv.reshape(8, 128).T
    ones = np.ones((128, 2), dtype=NP_BF16)

    counts = mask.sum(axis=1)
    K_T = int(np.ceil(counts.max() / 128))
    K_pad = K_T * 128
    n_half = (K_T + 1) // 2

    def tile_rows(xrows, nt):
        """[nt*128, D] rows -> [nt, 128, D] with [t, p, dc*128+c] =
        xrows[t*128+c, dc*128+p] (pre-tiled lhsT layout)."""
        return np.ascontiguousarray(
            xrows.reshape(nt, 128, 8, 128).transpose(0, 3, 2, 1)
            .reshape(nt, 128, D).astype(NP_BF16))

    in_maps = []
    for c in range(N_CORES):
        b, h = divmod(c, 2)
        sel = np.where(mask[b])[0]
        K = len(sel)
        xk = np.zeros((K_pad, D), dtype=np.float32)
        xk[:K] = x[b, sel]
        mb = np.zeros(K_pad, dtype=np.float32)
        mb[K:] = MASK_NEG
        cstv = np.concatenate(
            [aColT, np.ones((128, 8)), mb.reshape(K_T, 128).T], axis=1)
        xq_c = np.ascontiguousarray(
            x[b, h * QL:(h + 1) * QL].T.astype(NP_BF16))
        # This core's V share: its 512 dv columns (h-selected halves of
        # Wv^T and bv) over ALL packed keys.
        in_maps.append({
            "aP": aP, "xqT": xq_c, "xkP": tile_rows(xk, K_T),
            "wvH": np.ascontiguousarray(wvT_f[:, h * 512:(h + 1) * 512]),
            "bvb": np.ascontiguousarray(np.broadcast_to(
                bv[h * 512:(h + 1) * 512], (128, 512)).astype(NP_BF16)),
            "cst": np.ascontiguousarray(cstv.astype(np.float32)),
            "onesd": ones,
        })
    return in_maps, K_T


def run(x, mask, Wq, bq, Wk, bk, Wv, bv, trace=False):
    """Build + run; returns (output, BassKernelResults)."""
    in_maps, K_T = _prep_inputs(x, mask, Wq, bq, Wk, bk, Wv, bv)
    nc = _build_nc(K_T)
    res = run_bass_kernel_spmd(nc, in_maps, list(range(N_CORES)), trace=trace)
    out = np.empty((4, 2048, D), dtype=np.float32)
    for c in range(N_CORES):
        b, h = divmod(c, 2)
        out[b, h * QL:(h + 1) * QL, :] = res.results[c]["out"]
    return out, res


def kernel(x, mask, Wq, bq, Wk, bk, Wv, bv):
    out, _ = run(x, mask, Wq, bq, Wk, bk, Wv, bv)
    return out
